# revision 1
# baseline (speedup 1.0000x reference)
"""Trainium2 Bass kernel for CronRootAttention (sparse attention).

Shapes (hardcoded): B=2 H=16 S=4096 D=128, W=64, NB=R=64.
Sharding: fused B*H=32 axis split across 8 cores (4 slices/core).

v2 design, per (b,h) slice, per 128-query tile i:
  scores[j, n] PSUM layout: [local lw | interleaved (str,rel) pairs 2n]
    lw = 192 (128 for tile 0), n = 2i+1 valid strided/relay columns
  - QK: q-stationary matmuls vs kT window / interleaved kTsr prefix
  - local mask via identity-stationary matmul; str/rel boundary (4 cols)
    via one small DVE add with a fixed mask constant
  - ACT exp (scale folded) -> p bf16
  - PE transposes of p -> pT, one DVE copyout
  - PV matmuls vs v tiles augmented with a ones-column: out[:, 128] = row
    sums (joint softmax denominator) for free
  - DVE reciprocal + per-partition scale, DMA out (fp32)
  - emission is software-pipelined depth-2 so the PE never waits on
    ACT/DVE inside a tile
"""

import numpy as np
import ml_dtypes

import concourse.bass as bass
import concourse.bacc as bacc
import concourse.tile as tile
from concourse import mybir
from concourse.bass_utils import run_bass_kernel_spmd

BF16 = ml_dtypes.bfloat16
B, H, S, D = 2, 16, 4096, 128
W = 64
NB = S // W          # 64
R = NB               # 64
NCORES = 8
SLICES = B * H // NCORES   # 4
NT = S // 128        # 32 query tiles per slice
NEG = np.float32(-1e30)
SCALE = 1.0 / np.sqrt(np.float32(D))
DV = D + 1           # v columns + ones column

_prog_cache = {}


def _build_masks():
    j = np.arange(128)[:, None]
    c = np.arange(192)[None, :]
    mloc = np.where((c >= j + 1) & (c <= j + 64), 0.0, NEG).astype(np.float32)
    c0 = np.arange(128)[None, :]
    mloc0 = np.where((c0 >= np.maximum(j - 63, 0)) & (c0 <= j), 0.0, NEG).astype(np.float32)
    # sliding strided/relay masks, interleaved (str, rel) per column c:
    # column c of M_* corresponds to s - 2i + 64 (r - 2i + 64).
    cc = np.arange(128)[None, :]
    p = np.arange(128)[:, None]
    mstr = np.where(cc < 64 + (p >= 64).astype(np.int64), 1.0, 0.0).astype(np.float32)
    mrel = np.where(p >= 64 * (cc - 64) + 127, 1.0, 0.0).astype(np.float32)
    msr_int = np.empty((128, 256), np.float32)
    msr_int[:, 0::2] = mstr
    msr_int[:, 1::2] = mrel
    return mloc.astype(BF16), mloc0.astype(BF16), msr_int.astype(BF16)


def build_program():
    if "nc" in _prog_cache:
        return _prog_cache["nc"]
    dt = mybir.dt
    nc = bacc.Bacc("TRN2", target_bir_lowering=False, debug=False)

    qT_d = nc.declare_dram_parameter("qT", [SLICES, 128, S], dt.bfloat16, isOutput=False)
    kT_d = nc.declare_dram_parameter("kT", [SLICES, 128, S], dt.bfloat16, isOutput=False)
    vsh_d = nc.declare_dram_parameter("vsh", [SLICES, 128, 33 * DV], dt.bfloat16, isOutput=False)
    kTsr_d = nc.declare_dram_parameter("kTsr", [SLICES, 128, 128], dt.bfloat16, isOutput=False)
    vsr_d = nc.declare_dram_parameter("vsr", [SLICES, 128, DV], dt.bfloat16, isOutput=False)
    vn0_d = nc.declare_dram_parameter("vn0", [SLICES, 128, DV], dt.bfloat16, isOutput=False)
    ident_d = nc.declare_dram_parameter("ident", [128, 128], dt.bfloat16, isOutput=False)
    mloc_d = nc.declare_dram_parameter("mloc", [128, 192], dt.bfloat16, isOutput=False)
    mloc0_d = nc.declare_dram_parameter("mloc0", [128, 128], dt.bfloat16, isOutput=False)
    msr_d = nc.declare_dram_parameter("msr", [128, 256], dt.bfloat16, isOutput=False)
    out_d = nc.declare_dram_parameter("out", [SLICES, S, D], dt.float32, isOutput=True)

    from contextlib import ExitStack
    with tile.TileContext(nc) as tc, ExitStack() as ctx:
        cpool = ctx.enter_context(tc.tile_pool(name="consts", bufs=1))
        ident = cpool.tile([128, 128], dt.bfloat16, tag="ident")
        nc.sync.dma_start(ident[:], ident_d[:, :])
        mloc = cpool.tile([128, 192], dt.bfloat16, tag="mloc")
        nc.sync.dma_start(mloc[:], mloc_d[:, :])
        mloc0 = cpool.tile([128, 128], dt.bfloat16, tag="mloc0")
        nc.sync.dma_start(mloc0[:], mloc0_d[:, :])
        msr = cpool.tile([128, 256], dt.bfloat16, tag="msr")
        nc.sync.dma_start(msr[:], msr_d[:, :])

        spool = ctx.enter_context(tc.tile_pool(name="slice_in", bufs=2))
        pscores = ctx.enter_context(tc.tile_pool(name="pscores", bufs=3, space="PSUM"))
        ppt = ctx.enter_context(tc.tile_pool(name="ppt", bufs=2, space="PSUM"))
        pout = ctx.enter_context(tc.tile_pool(name="pout", bufs=2, space="PSUM"))
        wpool = ctx.enter_context(tc.tile_pool(name="work", bufs=3))
        ptpool = ctx.enter_context(tc.tile_pool(name="ptw", bufs=2))

        state = {}   # per in-flight tile: (i, scores, p_sb, tw, n)
        cur = {}

        def front(i):
            lw = 128 if i == 0 else 192
            n = 2 * i + 1
            tw = lw + 2 * n
            scores = pscores.tile([128, 320], dt.float32, tag="scores")
            qTi = cur["qT"][:, 128 * i:128 * (i + 1)]
            kloc = cur["kT"][:, 0:128] if i == 0 else cur["kT"][:, 128 * i - 64:128 * i + 128]
            mloc_i = mloc0[:, :] if i == 0 else mloc[:, :]
            nc.tensor.matmul(scores[:, 0:lw], qTi, kloc, start=True, stop=False)
            nc.tensor.matmul(scores[:, 0:lw], ident[:], mloc_i, start=False, stop=True)
            ksr_i = cur["kTsr"][:].rearrange("p (g c) -> p c g", g=2)[:, 0:n, :]
            nc.tensor.matmul(scores[:, lw:tw], qTi, ksr_i,
                             start=True, stop=True, skip_group_check=True)
            p_sb = wpool.tile([128, 320], dt.bfloat16, tag="p_sb")
            nc.scalar.activation(p_sb[:, 0:tw], scores[:, 0:tw],
                                 mybir.ActivationFunctionType.Exp, scale=float(SCALE))
            # boundary zeroing on p (post-exp; sums come from the PV
            # ones-column so they stay consistent): interleaved columns for
            # c in {2i-1, 2i} ({0} for tile 0), fixed slice of the sliding
            # 0/1 constant. Runs on the otherwise-idle GPSIMD.
            if i == 0:
                nc.gpsimd.tensor_mul(p_sb[:, lw:lw + 2], p_sb[:, lw:lw + 2],
                                     msr[:, 128:130])
            else:
                b0 = lw + 2 * (2 * i - 1)
                nc.gpsimd.tensor_mul(p_sb[:, b0:b0 + 4], p_sb[:, b0:b0 + 4],
                                     msr[:, 126:130])
            state[i] = (scores, p_sb, lw, n, tw, cur["vsh"], cur["vsr"], cur["vn0"], cur["out_s"])

        def back(i):
            scores, p_sb, lw, n, tw, vsh, vsr, vn0, out_s = state.pop(i)
            n2 = 2 * n
            ptp = ppt.tile([128, 384], dt.bfloat16, tag="ptp")
            nc.tensor.transpose(ptp[:, 0:128], p_sb[:, 0:128], ident[:])
            if i == 0:
                nc.tensor.transpose(ptp[0:n2, 128:256], p_sb[:, lw:tw], ident[:])
                cw = 256
            else:
                nc.tensor.transpose(ptp[0:64, 128:256], p_sb[:, 128:192], ident[:])
                nc.tensor.transpose(ptp[0:n2, 256:384], p_sb[:, lw:tw], ident[:])
                cw = 384
            pt = ptpool.tile([128, 384], dt.bfloat16, tag="pt")
            nc.vector.tensor_copy(pt[:, 0:cw], ptp[:, 0:cw])

            outp = pout.tile([128, DV], dt.float32, tag="outp")
            if i == 0:
                nc.tensor.matmul(outp[:], pt[:, 0:128], vn0[:], start=True, stop=False)
                nc.tensor.matmul(outp[:], pt[0:n2, 128:256], vsr[0:n2, :],
                                 start=False, stop=True)
            else:
                nc.tensor.matmul(outp[:], pt[:, 0:128],
                                 vsh[:, DV * i:DV * (i + 1)], start=True, stop=False)
                nc.tensor.matmul(outp[:], pt[0:64, 128:256],
                                 vsh[0:64, DV * (i + 1):DV * (i + 2)],
                                 start=False, stop=False)
                nc.tensor.matmul(outp[:], pt[0:n2, 256:384], vsr[0:n2, :],
                                 start=False, stop=True)

            rsum = wpool.tile([128, 1], dt.float32, tag="rsum")
            nc.vector.reciprocal(rsum[:], outp[:, 128:129])
            out_sb = wpool.tile([128, 128], dt.float32, tag="out_sb")
            nc.vector.tensor_scalar_mul(out_sb[:], outp[:, 0:128], rsum[:])
            nc.sync.dma_start(out_d[out_s, 128 * i:128 * (i + 1), :], out_sb[:])

        for s in range(SLICES):
            qT = spool.tile([128, S], dt.bfloat16, tag="qT")
            nc.sync.dma_start(qT[:], qT_d[s])
            kT = spool.tile([128, S], dt.bfloat16, tag="kT")
            nc.sync.dma_start(kT[:], kT_d[s])
            vsh = spool.tile([128, 33 * DV], dt.bfloat16, tag="vsh")
            nc.sync.dma_start(vsh[:], vsh_d[s])
            kTsr = spool.tile([128, 128], dt.bfloat16, tag="kTsr")
            nc.sync.dma_start(kTsr[:], kTsr_d[s])
            vsr = spool.tile([128, DV], dt.bfloat16, tag="vsr")
            nc.sync.dma_start(vsr[:], vsr_d[s])
            vn0 = spool.tile([128, DV], dt.bfloat16, tag="vn0")
            nc.sync.dma_start(vn0[:], vn0_d[s])
            cur.update(qT=qT, kT=kT, vsh=vsh, kTsr=kTsr, vsr=vsr, vn0=vn0, out_s=s)
            for i in range(NT):
                front(i)
                if i >= 2:
                    back(i - 2)
            back(NT - 2)
            back(NT - 1)

    nc.finalize()
    _prog_cache["nc"] = nc
    return nc


def _prep_core_inputs(q, k, v, rk, rv, masks, ident):
    """q,k,v: [SLICES, S, D] fp32 for one core; rk, rv: [SLICES, R, D]."""
    mloc, mloc0, msr = masks
    qb = q.astype(BF16)
    kb = k.astype(BF16)
    vb = v.astype(BF16)
    qT = np.ascontiguousarray(qb.transpose(0, 2, 1))          # [SL, 128, S]
    kT = np.ascontiguousarray(kb.transpose(0, 2, 1))
    # 64-shifted padded v tiles augmented with a ones column, stored
    # per-partition-contiguous: [SL, 128, 33*DV]
    vpad = np.concatenate([np.zeros((SLICES, 64, D), BF16), vb,
                           np.zeros((SLICES, 64, D), BF16)], axis=1)  # [SL, 4224, D]
    vpad = np.concatenate([vpad, np.ones((SLICES, 33 * 128, 1), BF16)], axis=2)
    vsh = np.ascontiguousarray(
        vpad.reshape(SLICES, 33, 128, DV).transpose(0, 2, 1, 3).reshape(SLICES, 128, 33 * DV))
    ksr = np.concatenate([kb[:, ::W, :], rk.astype(BF16)], axis=1)    # [SL, 128, D]
    kTsr = np.ascontiguousarray(ksr.transpose(0, 2, 1))               # [SL, 128, 128]
    # interleaved [str0, rel0, str1, rel1, ...] + ones column
    vsr_pairs = np.empty((SLICES, 128, D), BF16)
    vsr_pairs[:, 0::2] = vb[:, ::W, :]
    vsr_pairs[:, 1::2] = rv.astype(BF16)
    vsr = np.ascontiguousarray(
        np.concatenate([vsr_pairs, np.ones((SLICES, 128, 1), BF16)], axis=2))
    vn0 = np.ascontiguousarray(
        np.concatenate([vb[:, 0:128, :], np.ones((SLICES, 128, 1), BF16)], axis=2))
    return {
        "qT": qT, "kT": kT, "vsh": vsh, "kTsr": kTsr, "vsr": vsr, "vn0": vn0,
        "ident": ident, "mloc": mloc, "mloc0": mloc0, "msr": msr,
    }


def make_in_maps(q, k, v, rk, rv):
    masks = _build_masks()
    ident = np.eye(128, dtype=BF16)
    qf = q.reshape(B * H, S, D)
    kf = k.reshape(B * H, S, D)
    vf = v.reshape(B * H, S, D)
    rkf = rk.reshape(B * H, R, D)
    rvf = rv.reshape(B * H, R, D)
    in_maps = []
    for c in range(NCORES):
        sl = slice(SLICES * c, SLICES * (c + 1))
        in_maps.append(_prep_core_inputs(qf[sl], kf[sl], vf[sl], rkf[sl], rvf[sl],
                                         masks, ident))
    return in_maps


def kernel(q, k, v, rk, rv, _run_kwargs=None):
    q = np.asarray(q, dtype=np.float32)
    k = np.asarray(k, dtype=np.float32)
    v = np.asarray(v, dtype=np.float32)
    rk = np.asarray(rk, dtype=np.float32)
    rv = np.asarray(rv, dtype=np.float32)
    nc = build_program()
    in_maps = make_in_maps(q, k, v, rk, rv)
    res = run_bass_kernel_spmd(nc, in_maps, list(range(NCORES)), **(_run_kwargs or {}))
    out = np.stack([res.results[c]["out"] for c in range(NCORES)])  # [8, SL, S, D]
    if _run_kwargs:
        kernel.last_results = res
    return out.reshape(B, H, S, D)



# revision 9
# speedup vs baseline: 1.0845x; 1.0845x over previous
"""Trainium2 Bass kernel for CronRootAttention (sparse attention).

Shapes (hardcoded): B=2 H=16 S=4096 D=128, W=64, NB=R=64.
Sharding: fused B*H=32 axis split across 8 cores (4 slices/core).

v3 design: scores are computed TRANSPOSED (sT[key, query]) so the exp
output is already in the layout PV needs as its stationary operand.
Per (b,h) slice:
  - local QK: one matmul per 128-key block: sT[128k, 192q]
    (key block stationary, queries moving); the causal window mask is a
    single constant [128,192] 0/1 multiply, identical for every block
  - strided+relay QK: batched into 8 matmuls [128sr, 512q] per slice
    (kTsr stationary is tile-independent); boundary masks are tiny
    [3,128] 0/1 multiplies per query tile
  - ACT exp (scale folded) -> pT bf16 straight into SBUF; no PE
    transposes, no mask matmuls, no PSUM->SBUF copy
  - PV per 128-query tile: 3 accumulating matmuls (block i, tail of
    block i-1 into partitions 0:64, sr keys) vs v tiles augmented with
    a ones-column so out[:,128] is the joint softmax denominator
  - DVE reciprocal + per-partition scale, DMA out (fp32)
  - emission is software-pipelined depth-2 so the PE never waits on
    ACT/GPSIMD inside a tile
"""

import numpy as np
import ml_dtypes

import concourse.bass as bass
import concourse.bacc as bacc
import concourse.tile as tile
from concourse import mybir
from concourse.bass_utils import run_bass_kernel_spmd

BF16 = ml_dtypes.bfloat16
B, H, S, D = 2, 16, 4096, 128
W = 64
NB = S // W          # 64
R = NB               # 64
NCORES = 8
SLICES = B * H // NCORES   # 4
NT = S // 128        # 32 query tiles (= key blocks) per slice
SCALE = 1.0 / np.sqrt(np.float32(D))
DV = D + 1           # v columns + ones column

_prog_cache = {}


def _build_masks():
    # band01[t, jj]: key t (within block b) valid for query jj (offset from
    # block start 128b) iff jj-63 <= t <= jj.  Same for every block.
    t = np.arange(128)[:, None]
    jj = np.arange(192)[None, :]
    band01 = ((jj - 63 <= t) & (t <= jj)).astype(np.float32)
    # global sr validity mask over (interleaved sr row p, absolute query m):
    #   str row p=2s valid iff m >= 64s+64 = 32p+64
    #   rel row p=2s+1 valid iff m >= 64s+127 = 32(p-1)+127 = 32p+95
    p = np.arange(128)[:, None]
    m = np.arange(S)[None, :]
    thr = 32 * p + np.where(p % 2 == 0, 64, 95)
    srm01 = (m >= thr).astype(np.float32)
    return band01.astype(BF16), srm01.astype(BF16)


def build_program():
    if "nc" in _prog_cache:
        return _prog_cache["nc"]
    dt = mybir.dt
    nc = bacc.Bacc("TRN2", target_bir_lowering=False, debug=False)

    qT_d = nc.declare_dram_parameter("qT", [SLICES, 128, S], dt.bfloat16, isOutput=False)
    kT_d = nc.declare_dram_parameter("kT", [SLICES, 128, S], dt.bfloat16, isOutput=False)
    vb_d = nc.declare_dram_parameter("vb", [SLICES, 128, NT * DV], dt.bfloat16, isOutput=False)
    kTsr_d = nc.declare_dram_parameter("kTsr", [SLICES, 128, 128], dt.bfloat16, isOutput=False)
    vsr_d = nc.declare_dram_parameter("vsr", [SLICES, 128, DV], dt.bfloat16, isOutput=False)
    band_d = nc.declare_dram_parameter("band", [128, 192], dt.bfloat16, isOutput=False)
    srm_d = nc.declare_dram_parameter("srm", [128, S], dt.bfloat16, isOutput=False)
    out_d = nc.declare_dram_parameter("out", [SLICES, S, D], dt.float32, isOutput=True)

    from contextlib import ExitStack
    with tile.TileContext(nc) as tc, ExitStack() as ctx:
        cpool = ctx.enter_context(tc.tile_pool(name="consts", bufs=1))
        band = cpool.tile([128, 192], dt.bfloat16, tag="band")
        nc.sync.dma_start(band[:], band_d[:, :])
        srm = cpool.tile([128, S], dt.bfloat16, tag="srm")
        nc.sync.dma_start(srm[:], srm_d[:, :])

        spool = ctx.enter_context(tc.tile_pool(name="slice_in", bufs=2))
        srp = ctx.enter_context(tc.tile_pool(name="psrT", bufs=2))
        psr = ctx.enter_context(tc.tile_pool(name="psum_sr", bufs=2, space="PSUM"))
        ploc = ctx.enter_context(tc.tile_pool(name="psum_loc", bufs=3, space="PSUM"))
        pout = ctx.enter_context(tc.tile_pool(name="psum_out", bufs=3, space="PSUM"))
        plsb = ctx.enter_context(tc.tile_pool(name="p_sb", bufs=5))
        wout = ctx.enter_context(tc.tile_pool(name="wout", bufs=3))

        state = {}
        cur = {}

        def front(b):
            if b % 4 == 0:
                c = b // 4
                sch = psr.tile([128, 512], dt.float32, tag="sch")
                nc.tensor.matmul(sch[:], cur["kTsr"][:],
                                 cur["qT"][:, 512 * c:512 * (c + 1)],
                                 start=True, stop=True)
                nc.scalar.activation(cur["p_srT"][:, 512 * c:512 * (c + 1)], sch[:],
                                     mybir.ActivationFunctionType.Exp, scale=float(SCALE))
                nc.vector.tensor_mul(cur["p_srT"][:, 512 * c:512 * (c + 1)],
                                     cur["p_srT"][:, 512 * c:512 * (c + 1)],
                                     srm[:, 512 * c:512 * (c + 1)])
            qw = 192 if b < NT - 1 else 128
            sb_ = ploc.tile([128, 192], dt.float32, tag="sT")
            nc.tensor.matmul(sb_[:, 0:qw], cur["kT"][:, 128 * b:128 * b + 128],
                             cur["qT"][:, 128 * b:128 * b + qw], start=True, stop=True)
            p_b = plsb.tile([128, 192], dt.bfloat16, tag="p_b")
            nc.scalar.activation(p_b[:, 0:qw], sb_[:, 0:qw],
                                 mybir.ActivationFunctionType.Exp, scale=float(SCALE))
            nc.gpsimd.tensor_mul(p_b[:, 0:qw], p_b[:, 0:qw], band[:, 0:qw])
            state[b] = (p_b, state.get(b - 1, (None,))[0] if b > 0 else None,
                        cur["vb"], cur["vsr"], cur["p_srT"], cur["out_s"])

        def back(b):
            p_b, p_prev, vb, vsr, p_srT, out_s = state[b]
            outp = pout.tile([128, DV], dt.float32, tag="outp")
            nc.tensor.matmul(outp[:], p_b[:, 0:128], vb[:, DV * b:DV * (b + 1)],
                             start=True, stop=False)
            if b > 0:
                nc.tensor.matmul(outp[0:64, :], p_prev[:, 128:192],
                                 vb[:, DV * (b - 1):DV * b],
                                 start=False, stop=False, skip_group_check=True)
            n2 = 4 * b + 2
            nc.tensor.matmul(outp[:], p_srT[0:n2, 128 * b:128 * b + 128], vsr[0:n2, :],
                             start=False, stop=True, skip_group_check=True)
            rsum = wout.tile([128, 1], dt.float32, tag="rsum")
            nc.vector.reciprocal(rsum[:], outp[:, 128:129])
            out_sb = wout.tile([128, 128], dt.float32, tag="out_sb")
            nc.vector.tensor_scalar_mul(out_sb[:], outp[:, 0:128], rsum[:])
            nc.sync.dma_start(out_d[out_s, 128 * b:128 * (b + 1), :], out_sb[:])
            if b >= 2:
                state.pop(b - 2)

        for s in range(SLICES):
            qT = spool.tile([128, S], dt.bfloat16, tag="qT")
            nc.sync.dma_start(qT[:], qT_d[s])
            kT = spool.tile([128, S], dt.bfloat16, tag="kT")
            nc.sync.dma_start(kT[:], kT_d[s])
            vb = spool.tile([128, NT * DV], dt.bfloat16, tag="vb")
            nc.sync.dma_start(vb[:], vb_d[s])
            kTsr = spool.tile([128, 128], dt.bfloat16, tag="kTsr")
            nc.sync.dma_start(kTsr[:], kTsr_d[s])
            vsr = spool.tile([128, DV], dt.bfloat16, tag="vsr")
            nc.sync.dma_start(vsr[:], vsr_d[s])
            p_srT = srp.tile([128, S], dt.bfloat16, tag="p_srT")
            cur.update(qT=qT, kT=kT, vb=vb, kTsr=kTsr, vsr=vsr, p_srT=p_srT, out_s=s)
            for b in range(NT):
                front(b)
                if b >= 2:
                    back(b - 2)
            back(NT - 2)
            back(NT - 1)
            state.clear()

    nc.finalize()
    _prog_cache["nc"] = nc
    return nc


def _prep_core_inputs(q, k, v, rk, rv, masks):
    """q,k,v: [SLICES, S, D] fp32 for one core; rk, rv: [SLICES, R, D]."""
    band01, srm01 = masks
    qb = q.astype(BF16)
    kb = k.astype(BF16)
    vf = v.astype(BF16)
    qT = np.ascontiguousarray(qb.transpose(0, 2, 1))          # [SL, 128, S]
    kT = np.ascontiguousarray(kb.transpose(0, 2, 1))
    # blocked v with ones column, key-partition layout: vb[s, t, b*DV+d]
    vblk = vf.reshape(SLICES, NT, 128, D).transpose(0, 2, 1, 3)   # [SL,128,NT,D]
    vblk = np.concatenate([vblk, np.ones((SLICES, 128, NT, 1), BF16)], axis=3)
    vb = np.ascontiguousarray(vblk.reshape(SLICES, 128, NT * DV))
    # interleaved strided/relay keys: row 2j = k[64j], row 2j+1 = rk[j]
    ksr = np.empty((SLICES, 128, D), BF16)
    ksr[:, 0::2] = kb[:, ::W, :]
    ksr[:, 1::2] = rk.astype(BF16)
    kTsr = np.ascontiguousarray(ksr.transpose(0, 2, 1))           # [SL, 128, 128]
    vsr_pairs = np.empty((SLICES, 128, D), BF16)
    vsr_pairs[:, 0::2] = vf[:, ::W, :]
    vsr_pairs[:, 1::2] = rv.astype(BF16)
    vsr = np.ascontiguousarray(
        np.concatenate([vsr_pairs, np.ones((SLICES, 128, 1), BF16)], axis=2))
    return {
        "qT": qT, "kT": kT, "vb": vb, "kTsr": kTsr, "vsr": vsr,
        "band": band01, "srm": srm01,
    }


def make_in_maps(q, k, v, rk, rv):
    masks = _build_masks()
    qf = q.reshape(B * H, S, D)
    kf = k.reshape(B * H, S, D)
    vf = v.reshape(B * H, S, D)
    rkf = rk.reshape(B * H, R, D)
    rvf = rv.reshape(B * H, R, D)
    in_maps = []
    for c in range(NCORES):
        sl = slice(SLICES * c, SLICES * (c + 1))
        in_maps.append(_prep_core_inputs(qf[sl], kf[sl], vf[sl], rkf[sl], rvf[sl],
                                         masks))
    return in_maps


def kernel(q, k, v, rk, rv, _run_kwargs=None):
    q = np.asarray(q, dtype=np.float32)
    k = np.asarray(k, dtype=np.float32)
    v = np.asarray(v, dtype=np.float32)
    rk = np.asarray(rk, dtype=np.float32)
    rv = np.asarray(rv, dtype=np.float32)
    nc = build_program()
    in_maps = make_in_maps(q, k, v, rk, rv)
    res = run_bass_kernel_spmd(nc, in_maps, list(range(NCORES)), **(_run_kwargs or {}))
    out = np.stack([res.results[c]["out"] for c in range(NCORES)])  # [8, SL, S, D]
    if _run_kwargs:
        kernel.last_results = res
    return out.reshape(B, H, S, D)


# revision 17
# speedup vs baseline: 1.3784x; 1.2710x over previous
"""Trainium2 Bass kernel for CronRootAttention (sparse attention).

Shapes (hardcoded): B=2 H=16 S=4096 D=128, W=64, NB=R=64.
Sharding: fused B*H=32 axis split across 8 cores (4 slices/core).

v3 design: scores are computed TRANSPOSED (sT[key, query]) so the exp
output is already in the layout PV needs as its stationary operand.
Per (b,h) slice:
  - local QK: one matmul per 128-key block: sT[128k, 192q]
    (key block stationary, queries moving); the causal window mask is a
    single constant [128,192] 0/1 multiply, identical for every block
  - strided+relay QK: batched into 8 matmuls [128sr, 512q] per slice
    (kTsr stationary is tile-independent); boundary masks are tiny
    [3,128] 0/1 multiplies per query tile
  - ACT exp (scale folded) -> pT bf16 straight into SBUF; no PE
    transposes, no mask matmuls, no PSUM->SBUF copy
  - PV per 128-query tile: 3 accumulating matmuls (block i, tail of
    block i-1 into partitions 0:64, sr keys) vs v tiles augmented with
    a ones-column so out[:,128] is the joint softmax denominator
  - DVE reciprocal + per-partition scale, DMA out (fp32)
  - emission is software-pipelined depth-2 so the PE never waits on
    ACT/GPSIMD inside a tile
"""

import numpy as np
import ml_dtypes

import concourse.bass as bass
import concourse.bacc as bacc
import concourse.tile as tile
from concourse import mybir
from concourse.bass_utils import run_bass_kernel_spmd

BF16 = ml_dtypes.bfloat16
B, H, S, D = 2, 16, 4096, 128
W = 64
NB = S // W          # 64
R = NB               # 64
NCORES = 8
SLICES = B * H // NCORES   # 4
NT = S // 128        # 32 query tiles (= key blocks) per slice
SCALE = 1.0 / np.sqrt(np.float32(D))
DV = D + 1           # v columns + ones column

_prog_cache = {}


def _build_masks():
    # band01[t, jj]: key t (within block b) valid for query jj (offset from
    # block start 128b) iff jj-63 <= t <= jj.  Same for every block.
    t = np.arange(128)[:, None]
    jj = np.arange(192)[None, :]
    band01 = ((jj - 63 <= t) & (t <= jj)).astype(np.float32)
    # global sr validity mask over (interleaved sr row p, absolute query m):
    #   str row p=2s valid iff m >= 64s+64 = 32p+64
    #   rel row p=2s+1 valid iff m >= 64s+127 = 32(p-1)+127 = 32p+95
    p = np.arange(128)[:, None]
    m = np.arange(S)[None, :]
    thr = 32 * p + np.where(p % 2 == 0, 64, 95)
    srm01 = (m >= thr).astype(np.float32)
    return band01.astype(BF16), srm01.astype(BF16)


def build_program():
    if "nc" in _prog_cache:
        return _prog_cache["nc"]
    dt = mybir.dt
    nc = bacc.Bacc("TRN2", target_bir_lowering=False, debug=False)

    qT_d = nc.declare_dram_parameter("qT", [SLICES, 128, S], dt.bfloat16, isOutput=False)
    kT_d = nc.declare_dram_parameter("kT", [SLICES, 128, S], dt.bfloat16, isOutput=False)
    vb_d = nc.declare_dram_parameter("vb", [SLICES, 128, NT * DV], dt.bfloat16, isOutput=False)
    kTsr_d = nc.declare_dram_parameter("kTsr", [SLICES, 128, 128], dt.bfloat16, isOutput=False)
    vsr_d = nc.declare_dram_parameter("vsr", [SLICES, 128, DV], dt.bfloat16, isOutput=False)
    band_d = nc.declare_dram_parameter("band", [128, 192], dt.bfloat16, isOutput=False)
    srm_d = nc.declare_dram_parameter("srm", [128, S], dt.bfloat16, isOutput=False)
    # out[s, c, p, 128*t + d] = O[s, 512*c + 128*t + p, d]; un-permuted on host
    out_d = nc.declare_dram_parameter("out", [SLICES, 8, 128, 512], dt.float32, isOutput=True)

    from contextlib import ExitStack
    with tile.TileContext(nc) as tc, ExitStack() as ctx:
        cpool = ctx.enter_context(tc.tile_pool(name="consts", bufs=1))
        band = cpool.tile([128, 192], dt.bfloat16, tag="band")
        nc.sync.dma_start(band[:], band_d[:, :])
        srm = cpool.tile([128, S], dt.bfloat16, tag="srm")
        nc.sync.dma_start(srm[:], srm_d[:, :])

        spool = ctx.enter_context(tc.tile_pool(name="slice_in", bufs=2))
        srp = ctx.enter_context(tc.tile_pool(name="psrT", bufs=2))
        psr = ctx.enter_context(tc.tile_pool(name="psum_sr", bufs=2, space="PSUM"))
        ploc = ctx.enter_context(tc.tile_pool(name="psum_loc", bufs=3, space="PSUM"))
        pout = ctx.enter_context(tc.tile_pool(name="psum_out", bufs=3, space="PSUM"))
        plsb = ctx.enter_context(tc.tile_pool(name="p_sb", bufs=5))
        wout = ctx.enter_context(tc.tile_pool(name="wout", bufs=2))

        state = {}
        cur = {}

        def front(b):
            if b % 4 == 0:
                c = b // 4
                sch = psr.tile([128, 512], dt.float32, tag="sch")
                nc.tensor.matmul(sch[:], cur["kTsr"][:],
                                 cur["qT"][:, 512 * c:512 * (c + 1)],
                                 start=True, stop=True)
                nc.scalar.activation(cur["p_srT"][:, 512 * c:512 * (c + 1)], sch[:],
                                     mybir.ActivationFunctionType.Exp, scale=float(SCALE))
                nc.vector.tensor_mul(cur["p_srT"][:, 512 * c:512 * (c + 1)],
                                     cur["p_srT"][:, 512 * c:512 * (c + 1)],
                                     srm[:, 512 * c:512 * (c + 1)])
            qw = 192 if b < NT - 1 else 128
            sb_ = ploc.tile([128, 192], dt.float32, tag="sT")
            nc.tensor.matmul(sb_[:, 0:qw], cur["kT"][:, 128 * b:128 * b + 128],
                             cur["qT"][:, 128 * b:128 * b + qw], start=True, stop=True)
            p_b = plsb.tile([128, 192], dt.bfloat16, tag="p_b")
            nc.scalar.activation(p_b[:, 0:qw], sb_[:, 0:qw],
                                 mybir.ActivationFunctionType.Exp, scale=float(SCALE))
            nc.gpsimd.tensor_mul(p_b[:, 0:qw], p_b[:, 0:qw], band[:, 0:qw])
            state[b] = (p_b, state.get(b - 1, (None,))[0] if b > 0 else None,
                        cur["vb"], cur["vsr"], cur["p_srT"], cur["out_s"])

        def back(b):
            p_b, p_prev, vb, vsr, p_srT, out_s = state[b]
            outp = pout.tile([128, DV], dt.float32, tag="outp")
            nc.tensor.matmul(outp[:], p_b[:, 0:128], vb[:, DV * b:DV * (b + 1)],
                             start=True, stop=False)
            if b > 0:
                nc.tensor.matmul(outp[0:64, :], p_prev[:, 128:192],
                                 vb[:, DV * (b - 1):DV * b],
                                 start=False, stop=False, skip_group_check=True)
            n2 = 4 * b + 2
            nc.tensor.matmul(outp[:], p_srT[0:n2, 128 * b:128 * b + 128], vsr[0:n2, :],
                             start=False, stop=True, skip_group_check=True)
            rsum = wout.tile([128, 1], dt.float32, tag="rsum")
            nc.vector.reciprocal(rsum[:], outp[:, 128:129])
            t = b % 4
            if t == 0:
                ostage = wout.tile([128, 512], dt.float32, tag="ostage")
                cur["ostage"] = ostage
            ostage = cur["ostage"]
            nc.vector.tensor_scalar_mul(ostage[:, 128 * t:128 * (t + 1)],
                                        outp[:, 0:128], rsum[:])
            if t == 3:
                nc.sync.dma_start(out_d[out_s, b // 4], ostage[:])
            if b >= 2:
                state.pop(b - 2)

        def alloc_slice():
            qT = spool.tile([128, S], dt.bfloat16, tag="qT")
            kT = spool.tile([128, S], dt.bfloat16, tag="kT")
            vb = spool.tile([128, NT * DV], dt.bfloat16, tag="vb")
            kTsr = spool.tile([128, 128], dt.bfloat16, tag="kTsr")
            vsr = spool.tile([128, DV], dt.bfloat16, tag="vsr")
            p_srT = srp.tile([128, S], dt.bfloat16, tag="p_srT")
            return dict(qT=qT, kT=kT, vb=vb, kTsr=kTsr, vsr=vsr, p_srT=p_srT)

        def slice_dmas(s, t):
            d = [(t["kTsr"][:], kTsr_d[s]),
                 (t["qT"][:, 0:1024], qT_d[s, :, 0:1024]),
                 (t["kT"][:, 0:1024], kT_d[s, :, 0:1024]),
                 (t["vsr"][:], vsr_d[s]),
                 (t["vb"][:, 0:16 * DV], vb_d[s, :, 0:16 * DV])]
            for c0 in (1024, 2048, 3072):
                d.append((t["qT"][:, c0:c0 + 1024], qT_d[s, :, c0:c0 + 1024]))
                d.append((t["kT"][:, c0:c0 + 1024], kT_d[s, :, c0:c0 + 1024]))
            d.append((t["vb"][:, 16 * DV:NT * DV], vb_d[s, :, 16 * DV:NT * DV]))
            return d

        nxt = alloc_slice()
        for dst, src in slice_dmas(0, nxt):
            nc.sync.dma_start(dst, src)
        for s in range(SLICES):
            cur.clear()
            cur.update(nxt, out_s=s)
            pend = []
            if s + 1 < SLICES:
                nxt = alloc_slice()
                pend = slice_dmas(s + 1, nxt)
            for b in range(NT):
                front(b)
                if 4 <= b < 4 + len(pend):
                    dst, src = pend[b - 4]
                    nc.sync.dma_start(dst, src)
                if b >= 2:
                    back(b - 2)
            back(NT - 2)
            back(NT - 1)
            state.clear()

    nc.finalize()
    _prog_cache["nc"] = nc
    return nc


def _prep_core_inputs(q, k, v, rk, rv, masks):
    """q,k,v: [SLICES, S, D] fp32 for one core; rk, rv: [SLICES, R, D]."""
    band01, srm01 = masks
    qb = q.astype(BF16)
    kb = k.astype(BF16)
    vf = v.astype(BF16)
    qT = np.ascontiguousarray(qb.transpose(0, 2, 1))          # [SL, 128, S]
    kT = np.ascontiguousarray(kb.transpose(0, 2, 1))
    # blocked v with ones column, key-partition layout: vb[s, t, b*DV+d]
    vblk = vf.reshape(SLICES, NT, 128, D).transpose(0, 2, 1, 3)   # [SL,128,NT,D]
    vblk = np.concatenate([vblk, np.ones((SLICES, 128, NT, 1), BF16)], axis=3)
    vb = np.ascontiguousarray(vblk.reshape(SLICES, 128, NT * DV))
    # interleaved strided/relay keys: row 2j = k[64j], row 2j+1 = rk[j]
    ksr = np.empty((SLICES, 128, D), BF16)
    ksr[:, 0::2] = kb[:, ::W, :]
    ksr[:, 1::2] = rk.astype(BF16)
    kTsr = np.ascontiguousarray(ksr.transpose(0, 2, 1))           # [SL, 128, 128]
    vsr_pairs = np.empty((SLICES, 128, D), BF16)
    vsr_pairs[:, 0::2] = vf[:, ::W, :]
    vsr_pairs[:, 1::2] = rv.astype(BF16)
    vsr = np.ascontiguousarray(
        np.concatenate([vsr_pairs, np.ones((SLICES, 128, 1), BF16)], axis=2))
    return {
        "qT": qT, "kT": kT, "vb": vb, "kTsr": kTsr, "vsr": vsr,
        "band": band01, "srm": srm01,
    }


def make_in_maps(q, k, v, rk, rv):
    masks = _build_masks()
    qf = q.reshape(B * H, S, D)
    kf = k.reshape(B * H, S, D)
    vf = v.reshape(B * H, S, D)
    rkf = rk.reshape(B * H, R, D)
    rvf = rv.reshape(B * H, R, D)
    in_maps = []
    for c in range(NCORES):
        sl = slice(SLICES * c, SLICES * (c + 1))
        in_maps.append(_prep_core_inputs(qf[sl], kf[sl], vf[sl], rkf[sl], rvf[sl],
                                         masks))
    return in_maps


def kernel(q, k, v, rk, rv, _run_kwargs=None):
    q = np.asarray(q, dtype=np.float32)
    k = np.asarray(k, dtype=np.float32)
    v = np.asarray(v, dtype=np.float32)
    rk = np.asarray(rk, dtype=np.float32)
    rv = np.asarray(rv, dtype=np.float32)
    nc = build_program()
    in_maps = make_in_maps(q, k, v, rk, rv)
    res = run_bass_kernel_spmd(nc, in_maps, list(range(NCORES)), **(_run_kwargs or {}))
    out = np.stack([res.results[c]["out"] for c in range(NCORES)])  # [8, SL, 8, 128, 512]
    if _run_kwargs:
        kernel.last_results = res
    # out[core, s, c, p, 128*t + d] = O[core, s, 512*c + 128*t + p, d]
    out = out.reshape(NCORES, SLICES, 8, 128, 4, D).transpose(0, 1, 2, 4, 3, 5)
    return np.ascontiguousarray(out).reshape(B, H, S, D)
